# revision 35
# baseline (speedup 1.0000x reference)
"""Causal self-attention (B=2, S=2048, D=1024, H=16) on 8 TRN2 NeuronCores.

Sharding: tensor-parallel over heads. Core c owns heads {2c, 2c+1} for BOTH
batches: it computes Q/K/V projections for its 2 heads (1/8 of the QKV work,
no redundancy), causal attention for its heads over all tokens (skipping
fully-masked 128x256 blocks; diagonal blocks handled by a 0/1 mask multiply
after exp), and softmax-normalizes via a ones-column appended to V (row 64 of
the PV accumulator = denominators).

The normalized per-head outputs y^T are then redistributed with a single
all-to-all over the 8 cores (bf16, 1MB per core): core c sends
y^T[my 128 dims, tokens of core j] to each j, and receives its own 512
tokens' y for all 1024 dims. Each core then runs the output projection for
its 512 tokens locally and writes the final [512, 1024] f32 block.

All matmuls run in bf16 (full PE rate at any moving size); PSUM accumulates
f32. Engines execute in program order per engine, so the emission order
software-pipelines everything explicitly:
  phase 1: QKV(batch 0)
  phase 2: attention(batch 0, both heads round-robin, PV one tile behind
           QK/exp) with QKV(batch 1) sub-units interleaved every 3 tiles to
           fill PE gaps while ACT does exp
  phase 3: attention(batch 1) with batch-0 normalization interleaved
  phase 4: normalize(batch 1), all-to-all, output projection
"""

from collections import deque

import numpy as np

import concourse.bass as bass
import concourse.mybir as mybir
import concourse.tile as tile
from concourse import bacc
from concourse.bass_utils import run_bass_kernel_spmd

F32 = mybir.dt.float32
BF16 = mybir.dt.bfloat16
AF = mybir.ActivationFunctionType
ALU = mybir.AluOpType

B, S, D, H, HD = 2, 2048, 1024, 16, 64
QL = 512           # tokens output per core
NKC = D // 128     # 8 contraction chunks for the projections
QG = 256           # attention q-group width
NQG = S // QG      # 8 q-groups
TCH = 512          # token chunk width for streaming x^T
NTC = S // TCH     # 4
SCALE = 1.0 / np.sqrt(HD)

_CACHED = {}

NPBF16 = mybir.dt.np(BF16)


def build_nc():
    nc = bacc.Bacc("TRN2", target_bir_lowering=False, debug=False)

    xt0 = nc.dram_tensor("xt0", [D, S], BF16, kind="ExternalInput").ap()
    xt1 = nc.dram_tensor("xt1", [D, S], BF16, kind="ExternalInput").ap()
    wq = nc.dram_tensor("wq", [D, 128], BF16, kind="ExternalInput").ap()
    wk = nc.dram_tensor("wk", [D, 128], BF16, kind="ExternalInput").ap()
    wv = nc.dram_tensor("wv", [D, 128], BF16, kind="ExternalInput").ap()
    bq = nc.dram_tensor("bq", [128, 1], F32, kind="ExternalInput").ap()
    bk = nc.dram_tensor("bk", [128, 1], F32, kind="ExternalInput").ap()
    bv = nc.dram_tensor("bv", [1, 128], BF16, kind="ExternalInput").ap()
    wp = nc.dram_tensor("wp", [D, D], BF16, kind="ExternalInput").ap()
    bp = nc.dram_tensor("bp", [1, D], BF16, kind="ExternalInput").ap()
    maskd = nc.dram_tensor("maskd", [128, 2 * QG], BF16, kind="ExternalInput").ap()
    ones1 = nc.dram_tensor("ones1", [1, 128], BF16, kind="ExternalInput").ap()
    out = nc.dram_tensor("out", [QL, D], F32, kind="ExternalOutput").ap()

    with tile.TileContext(nc) as tc:
        _body(nc, tc, [xt0, xt1], wq, wk, wv, bq, bk, bv, wp, bp, maskd,
              ones1, out)
    nc.compile()
    return nc


class _Ctx:
    pass


def _body(nc, tc, xt, wq, wk, wv, bq, bk, bv, wp, bp, maskd, ones1, out):
    with (
        tc.tile_pool(name="const", bufs=1) as const_p,
        tc.tile_pool(name="w", bufs=1) as w_p,
        tc.tile_pool(name="qkv", bufs=1) as qkv_p,
        tc.tile_pool(name="xtc", bufs=2) as xtc_p,
        tc.tile_pool(name="pt", bufs=6) as p_p,
        tc.tile_pool(name="ys", bufs=16) as ys_p,
        tc.tile_pool(name="rec", bufs=4) as rec_p,
        tc.tile_pool(name="outp", bufs=3) as out_p,
        tc.tile_pool(name="psum", bufs=5, space="PSUM") as psum,
        tc.tile_pool(name="opsum", bufs=2, space="PSUM") as opsum,
        tc.tile_pool(name="rpsum", bufs=1, space="PSUM") as rpsum,
        tc.tile_pool(name="dram", bufs=1, space="DRAM") as dram,
    ):
        # ---------------- constants (tiles now, DMAs emitted just after
        # the first x-chunk DMA so phase 1 starts immediately) ----------
        ones_s = const_p.tile([1, 128], BF16)
        mask_s = const_p.tile([128, 2 * QG], BF16)
        bq_s = const_p.tile([128, 1], F32)
        bk_s = const_p.tile([128, 1], F32)
        bv_s = const_p.tile([1, 128], BF16)
        bp_s = const_p.tile([1, D], BF16)

        def emit_const_dmas():
            nc.sync.dma_start(ones_s[:], ones1[:])
            nc.sync.dma_start(bq_s[:], bq[:])
            nc.sync.dma_start(bk_s[:], bk[:])
            nc.sync.dma_start(bv_s[:], bv[:])
            nc.sync.dma_start(mask_s[:], maskd[:])
            nc.sync.dma_start(bp_s[:], bp[:])

        # ---------------- weights (one DMA each, gather-rearranged) -------
        wq_s = w_p.tile([128, D], BF16)   # [d-chunk part, kc*128 + qcol]
        wk_s = w_p.tile([128, D], BF16)
        wv_s = w_p.tile([128, D], BF16)
        for ws, wsrc in ((wq_s, wq), (wk_s, wk), (wv_s, wv)):
            nc.sync.dma_start(
                ws[:].rearrange("p (c e) -> p c e", c=NKC),
                wsrc[:].rearrange("(c p) e -> p c e", p=128),
            )

        cx = _Ctx()
        cx.qt = [[qkv_p.tile([128, TCH], BF16, tag=f"qt{b}_{t}",
                             name=f"qt{b}_{t}") for t in range(NTC)]
                 for b in range(B)]
        cx.kt = [[qkv_p.tile([128, TCH], BF16, tag=f"kt{b}_{t}",
                             name=f"kt{b}_{t}") for t in range(NTC)]
                 for b in range(B)]
        cx.v2 = [[qkv_p.tile([128, 4 * 130], BF16, tag=f"v{b}_{t}",
                             name=f"v{b}_{t}") for t in range(NTC)]
                 for b in range(B)]
        for b in range(B):
            for t in range(NTC):
                v4 = cx.v2[b][t][:].rearrange("p (t h e) -> p t h e",
                                              h=2, e=65)
                nc.vector.memset(v4[:, :, :, 64:65], 1.0)
        cx.ytn = [qkv_p.tile([128, S], BF16, tag=f"ytn{b}", name=f"ytn{b}")
                  for b in range(B)]
        cx.xc = {}

        # ---------- QKV sub-unit emitters ----------
        def qkv_dma(b, tcn):
            xc = xtc_p.tile([128, NKC * TCH], BF16, tag="x",
                            name=f"x{b}_{tcn}")
            nc.sync.dma_start(
                xc[:].rearrange("p (c t) -> p c t", c=NKC),
                xt[b][:].rearrange("(c p) t -> p c t", p=128)[
                    :, :, tcn * TCH:(tcn + 1) * TCH],
            )
            cx.xc[(b, tcn)] = xc

        def qkv_q(b, tcn):
            xc = cx.xc[(b, tcn)]
            ps = psum.tile([128, TCH], F32, tag="ps", name=f"q{b}_{tcn}")
            for kc in range(NKC):
                nc.tensor.matmul(
                    ps[:], wq_s[:, kc * 128:(kc + 1) * 128],
                    xc[:, kc * TCH:(kc + 1) * TCH],
                    start=(kc == 0), stop=(kc == NKC - 1),
                )
            if b == 0:   # batch 0: ACT is idle during this phase
                nc.scalar.activation(cx.qt[b][tcn][:], ps[:], AF.Identity,
                                     bias=bq_s[:])
            else:
                nc.vector.tensor_scalar(
                    cx.qt[b][tcn][:], ps[:], bq_s[:], None, ALU.add)

        def qkv_k(b, tcn):
            xc = cx.xc[(b, tcn)]
            ps = psum.tile([128, TCH], F32, tag="ps", name=f"k{b}_{tcn}")
            for kc in range(NKC):
                nc.tensor.matmul(
                    ps[:], wk_s[:, kc * 128:(kc + 1) * 128],
                    xc[:, kc * TCH:(kc + 1) * TCH],
                    start=(kc == 0), stop=(kc == NKC - 1),
                )
            if b == 0:
                nc.scalar.activation(cx.kt[b][tcn][:], ps[:], AF.Identity,
                                     bias=bk_s[:])
            else:
                nc.vector.tensor_scalar(
                    cx.kt[b][tcn][:], ps[:], bk_s[:], None, ALU.add)

        def qkv_v(b, tcn, vt):
            xc = cx.xc[(b, tcn)]
            ps = psum.tile([128, 512], F32, tag="ps", name=f"v{b}_{tcn}_{vt}")
            for kc in range(NKC):
                nc.tensor.matmul(
                    ps[:, 0:128],
                    xc[:, kc * TCH + vt * 128:kc * TCH + (vt + 1) * 128],
                    wv_s[:, kc * 128:(kc + 1) * 128],
                    start=(kc == 0), stop=False,
                )
            # fold the per-column v-bias in as a K=1 rank-1 update
            nc.tensor.matmul(
                ps[:, 0:128], ones_s[:], bv_s[:], start=False, stop=True,
            )
            v4 = cx.v2[b][tcn][:].rearrange("p (t h e) -> p t h e",
                                            h=2, e=65)
            if b == 0:
                nc.scalar.activation(
                    v4[:, vt, :, 0:64],
                    ps[:, 0:128].rearrange("p (h e) -> p h e", e=64),
                    AF.Copy,
                )
            else:
                nc.vector.tensor_copy(
                    v4[:, vt, :, 0:64],
                    ps[:, 0:128].rearrange("p (h e) -> p h e", e=64),
                )

        def qkv_units(b):
            units = []
            for tcn in range(NTC):
                units.append(lambda b=b, t=tcn: qkv_dma(b, t))
                units.append(lambda b=b, t=tcn: qkv_q(b, t))
                units.append(lambda b=b, t=tcn: qkv_k(b, t))
                for vt in range(4):
                    units.append(lambda b=b, t=tcn, v=vt: qkv_v(b, t, v))
            return deque(units)

        # ---------- attention stream (skewed: PV one tile behind) ----------
        class AttStream:
            def __init__(self, b, hh):
                self.b, self.hh = b, hh
                self.e0 = hh * 64
                self.ys = [ys_p.tile([65, 2 * QG], F32, tag="ys",
                                     name=f"ys{b}_{hh}_{gp}")
                           for gp in range(NQG // 2)]
                self.rec = rec_p.tile([1, S], BF16, tag="rec",
                                      name=f"rec{b}_{hh}")
                self.tiles = deque()
                for g in range(NQG):
                    npr = g + 1
                    # diagonal tile first: its extra mask hop hides behind
                    # the remaining full tiles' PV work
                    order = [npr - 1] + list(range(npr - 1))
                    for i, pr in enumerate(order):
                        self.tiles.append(
                            (g, pr, pr == npr - 1, i == 0, i == npr - 1))
                self.prev = None
                self.ops = {}
                self.gps_done = 0

            @property
            def has(self):
                return bool(self.tiles) or self.prev is not None

            def _emit_qk(self, g, pr, diag):
                b, hh, e0 = self.b, self.hh, self.e0
                q0 = g * QG
                sp = psum.tile([128, 2 * QG], F32, tag="ps",
                               name=f"s{b}_{hh}_{g}_{pr}")
                pt = p_p.tile([128, 2 * QG], BF16, tag="pt",
                              name=f"p{b}_{hh}_{g}_{pr}")
                for kk in range(2):
                    kvc = 2 * pr + kk
                    nc.tensor.matmul(
                        sp[:, kk * QG:(kk + 1) * QG],
                        cx.kt[b][kvc // 4][e0:e0 + 64,
                                           (kvc % 4) * 128:
                                           (kvc % 4 + 1) * 128],
                        cx.qt[b][g // 2][e0:e0 + 64,
                                         (q0 % TCH):(q0 % TCH) + QG],
                        start=True, stop=True,
                    )
                nc.scalar.activation(pt[:], sp[:], AF.Exp)
                if diag:
                    nc.vector.tensor_tensor(pt[:], pt[:], mask_s[:],
                                            ALU.mult)
                return pt

            def _emit_pv(self):
                g, pr, diag, first, last, ptp = self.prev
                b, hh = self.b, self.hh
                gp, gh = g // 2, g % 2
                if gp not in self.ops:
                    self.ops[gp] = opsum.tile([65, 2 * QG], F32, tag="o",
                                              name=f"o{b}_{hh}_{gp}")
                op = self.ops[gp]
                for kk in range(2):
                    kvc = 2 * pr + kk
                    v4 = cx.v2[b][kvc // 4][:].rearrange(
                        "p (t h e) -> p t h e", h=2, e=65)
                    nc.tensor.matmul(
                        op[:, gh * QG:(gh + 1) * QG],
                        v4[:, kvc % 4, hh, :],
                        ptp[:, kk * QG:(kk + 1) * QG],
                        start=(first and kk == 0),
                        stop=(last and kk == 1),
                    )
                if last and gh == 1:
                    nc.vector.tensor_copy(self.ys[gp][:], op[:])
                    del self.ops[gp]
                    self.gps_done += 1
                self.prev = None

            def step(self):
                """Emit QK+exp for the next tile, then PV for the previous."""
                if self.tiles:
                    g, pr, diag, first, last = self.tiles.popleft()
                    pt = self._emit_qk(g, pr, diag)
                    if self.prev is not None:
                        self._emit_pv()
                    self.prev = (g, pr, diag, first, last, pt)
                elif self.prev is not None:
                    self._emit_pv()

            def norm_q(self, m):
                """Normalize token slice [512m, 512m+512) (= ys tile m)."""
                b, hh, e0 = self.b, self.hh, self.e0
                rec = self.rec
                with nc.allow_low_precision(reason="bf16 softmax denom"):
                    nc.vector.reciprocal(
                        rec[:, m * 512:(m + 1) * 512],
                        self.ys[m][64:65, :])
                rp = rpsum.tile([64, 512], F32, tag="rp",
                                name=f"rp{b}_{hh}_{m}")
                nc.tensor.matmul(
                    rp[:], ones_s[:, 0:64],
                    rec[:, m * 512:(m + 1) * 512],
                    start=True, stop=True,
                )
                nc.vector.tensor_tensor(
                    cx.ytn[b][e0:e0 + 64, m * 512:(m + 1) * 512],
                    self.ys[m][0:64, :],
                    rp[:], ALU.mult,
                )

        a2a_in = dram.tile([8 * 128, QL], BF16)
        a2a_out = dram.tile([8 * 128, QL], BF16)

        def a2a_send(b, m):
            j = 4 * b + m
            nc.sync.dma_start(
                a2a_in[j * 128:(j + 1) * 128, :],
                cx.ytn[b][:, m * QL:(m + 1) * QL],
            )

        # ================= emission schedule =================
        # phase 1: QKV(b0) with early b0 attention tiles as they unblock
        s00, s01 = AttStream(0, 0), AttStream(0, 1)

        def tcn_req(st):
            g, pr = st.tiles[0][0], st.tiles[0][1]
            return max(g // 2, (2 * pr + 1) // 4)

        b0_units = qkv_units(0)
        b0_units.popleft()()      # x(b0, tcn0) DMA first
        emit_const_dmas()
        done_tcn0 = -1
        unit_i = 1
        while b0_units:
            b0_units.popleft()()
            unit_i += 1
            if unit_i % 7 == 0:
                done_tcn0 += 1   # a full tcn group (dma+q+k+4v) emitted
            for st in (s00, s01):
                if st.tiles and tcn_req(st) <= done_tcn0:
                    st.step()
        # phase 2: rest of attention(b0) + QKV(b1) interleave
        b1_units = qkv_units(1)
        tilesteps = 0
        while s00.has or s01.has:
            for st in (s00, s01):
                if st.has:
                    st.step()
                    tilesteps += 1
                    if tilesteps % 3 == 0 and b1_units:
                        b1_units.popleft()()
        while b1_units:
            b1_units.popleft()()
        # wp load: no deps; DMA queue is idle from here until the a2a sends
        wp_s = w_p.tile([128, NKC * D], BF16, tag="wp", name="wp")
        nc.sync.dma_start(
            wp_s[:].rearrange("p (c e) -> p c e", c=NKC),
            wp[:].rearrange("(c p) e -> p c e", p=128),
        )
        # phase 3: attention(b1); batch-0 norms+sends and progressive
        # batch-1 norms+sends interleave as their ys tiles complete
        s10, s11 = AttStream(1, 0), AttStream(1, 1)
        b0_norm = deque(
            [(s00, 0), (s01, 0), (0, 0),
             (s00, 1), (s01, 1), (0, 1),
             (s00, 2), (s01, 2), (0, 2),
             (s00, 3), (s01, 3), (0, 3)]
        )
        sent1 = 0
        steps = 0
        while s10.has or s11.has:
            for st in (s10, s11):
                if st.has:
                    st.step()
                    steps += 1
                    if steps % 4 == 0 and b0_norm:
                        job = b0_norm.popleft()
                        if isinstance(job[0], AttStream):
                            job[0].norm_q(job[1])
                        else:
                            a2a_send(0, job[1])
            while (sent1 < 3
                   and min(s10.gps_done, s11.gps_done) > sent1):
                s10.norm_q(sent1)
                s11.norm_q(sent1)
                a2a_send(1, sent1)
                sent1 += 1
        while b0_norm:
            job = b0_norm.popleft()
            if isinstance(job[0], AttStream):
                job[0].norm_q(job[1])
            else:
                a2a_send(0, job[1])
        # phase 4: last batch-1 slice, exchange, output projection
        for m in range(sent1, 4):
            s10.norm_q(m)
            s11.norm_q(m)
            a2a_send(1, m)

        nc.gpsimd.collective_compute(
            "AllToAll",
            ALU.bypass,
            replica_groups=[[0, 1, 2, 3, 4, 5, 6, 7]],
            ins=[a2a_in[:]],
            outs=[a2a_out[:]],
        )
        yg = qkv_p.tile([128, 8 * QL], BF16, tag="yg", name="yg")
        for r in range(8):
            nc.sync.dma_start(yg[:, r * QL:(r + 1) * QL],
                              a2a_out[r * 128:(r + 1) * 128, :])

        for tc4 in range(4):
            for n2 in range(2):
                ps = psum.tile([128, 512], F32, tag="ps",
                               name=f"op{tc4}_{n2}")
                for r in range(8):
                    nc.tensor.matmul(
                        ps[:],
                        yg[:, r * QL + tc4 * 128:r * QL + (tc4 + 1) * 128],
                        wp_s[:, r * D + n2 * 512:r * D + (n2 + 1) * 512],
                        start=(r == 0), stop=False,
                    )
                nc.tensor.matmul(
                    ps[:], ones_s[:], bp_s[:, n2 * 512:(n2 + 1) * 512],
                    start=False, stop=True,
                )
                ot = out_p.tile([128, 512], F32, tag="ot",
                                name=f"ot{tc4}_{n2}")
                nc.scalar.activation(ot[:], ps[:], AF.Copy)
                nc.sync.dma_start(
                    out[tc4 * 128:(tc4 + 1) * 128,
                        n2 * 512:(n2 + 1) * 512],
                    ot[:],
                )


def _host_inputs(x, w_attn, b_attn, w_proj, b_proj):
    """Build the 8 per-core input maps."""
    x = np.asarray(x, np.float32)
    w_attn = np.asarray(w_attn, np.float32)
    b_attn = np.asarray(b_attn, np.float32)
    w_proj = np.asarray(w_proj, np.float32)
    b_proj = np.asarray(b_proj, np.float32)

    xt0 = np.ascontiguousarray(x[0].T).astype(NPBF16)
    xt1 = np.ascontiguousarray(x[1].T).astype(NPBF16)
    wpp = np.ascontiguousarray(w_proj).astype(NPBF16)
    bp1 = b_proj.reshape(1, D).astype(NPBF16)
    ones1 = np.ones((1, 128), NPBF16)

    # diagonal-pair 0/1 mask: cols [0:256] kv-offset 0..127, [256:512] 128..255
    p = np.arange(128)[:, None]
    q = np.arange(QG)[None, :]
    maskd = np.concatenate(
        [(p <= q).astype(np.float32), (p + 128 <= q).astype(np.float32)],
        axis=1,
    ).astype(NPBF16)

    in_maps = []
    for c in range(8):
        c0 = 128 * c
        wqc = (w_attn[:, c0:c0 + 128] * SCALE).astype(NPBF16)
        wkc = w_attn[:, D + c0:D + c0 + 128].astype(NPBF16)
        wvc = w_attn[:, 2 * D + c0:2 * D + c0 + 128].astype(NPBF16)
        bqc = (b_attn[c0:c0 + 128] * SCALE).astype(np.float32).reshape(128, 1)
        bkc = b_attn[D + c0:D + c0 + 128].astype(np.float32).reshape(128, 1)
        bvc = b_attn[2 * D + c0:2 * D + c0 + 128].reshape(1, 128).astype(NPBF16)
        in_maps.append(
            {
                "xt0": xt0, "xt1": xt1,
                "wq": wqc, "wk": wkc, "wv": wvc,
                "bq": bqc, "bk": bkc, "bv": bvc,
                "wp": wpp, "bp": bp1,
                "maskd": maskd, "ones1": ones1,
            }
        )
    return in_maps


def _assemble_full(outs):
    full = np.empty((B, S, D), np.float32)
    for c in range(8):
        b, cq = c // 4, c % 4
        full[b, cq * QL:(cq + 1) * QL] = outs[c]
    return full


def kernel(x, w_attn, b_attn, w_proj, b_proj):
    if "nc" not in _CACHED:
        _CACHED["nc"] = build_nc()
    nc = _CACHED["nc"]
    in_maps = _host_inputs(x, w_attn, b_attn, w_proj, b_proj)
    res = run_bass_kernel_spmd(nc, in_maps, core_ids=list(range(8)))
    _CACHED["last_results"] = res
    outs = [res.results[c]["out"] for c in range(8)]
    return _assemble_full(outs)


# revision 39
# speedup vs baseline: 1.0127x; 1.0127x over previous
"""Causal self-attention (B=2, S=2048, D=1024, H=16) on 8 TRN2 NeuronCores.

Sharding: tensor-parallel over heads. Core c owns heads {2c, 2c+1} for BOTH
batches: it computes Q/K/V projections for its 2 heads (1/8 of the QKV work,
no redundancy), causal attention for its heads over all tokens (skipping
fully-masked 128x256 blocks; diagonal blocks handled by a 0/1 mask multiply
after exp), and softmax-normalizes via a ones-column appended to V (row 64 of
the PV accumulator = denominators).

The normalized per-head outputs y^T are then redistributed with a single
all-to-all over the 8 cores (bf16, 1MB per core): core c sends
y^T[my 128 dims, tokens of core j] to each j, and receives its own 512
tokens' y for all 1024 dims. Each core then runs the output projection for
its 512 tokens locally and writes the final [512, 1024] f32 block.

All matmuls run in bf16 (full PE rate at any moving size); PSUM accumulates
f32. Engines execute in program order per engine, so the emission order
software-pipelines everything explicitly:
  phase 1: QKV(batch 0)
  phase 2: attention(batch 0, both heads round-robin, PV one tile behind
           QK/exp) with QKV(batch 1) sub-units interleaved every 3 tiles to
           fill PE gaps while ACT does exp
  phase 3: attention(batch 1) with batch-0 normalization interleaved
  phase 4: normalize(batch 1), all-to-all, output projection
"""

from collections import deque

import numpy as np

import concourse.bass as bass
import concourse.mybir as mybir
import concourse.tile as tile
from concourse import bacc
from concourse.bass_utils import run_bass_kernel_spmd

F32 = mybir.dt.float32
BF16 = mybir.dt.bfloat16
AF = mybir.ActivationFunctionType
ALU = mybir.AluOpType

B, S, D, H, HD = 2, 2048, 1024, 16, 64
QL = 512           # tokens output per core
NKC = D // 128     # 8 contraction chunks for the projections
QG = 256           # attention q-group width
NQG = S // QG      # 8 q-groups
TCH = 512          # token chunk width for streaming x^T
NTC = S // TCH     # 4
SCALE = 1.0 / np.sqrt(HD)

_CACHED = {}

NPBF16 = mybir.dt.np(BF16)


def build_nc():
    nc = bacc.Bacc("TRN2", target_bir_lowering=False, debug=False)

    xt0 = nc.dram_tensor("xt0", [D, S], BF16, kind="ExternalInput").ap()
    xt1 = nc.dram_tensor("xt1", [D, S], BF16, kind="ExternalInput").ap()
    wq = nc.dram_tensor("wq", [D, 128], BF16, kind="ExternalInput").ap()
    wk = nc.dram_tensor("wk", [D, 128], BF16, kind="ExternalInput").ap()
    wv = nc.dram_tensor("wv", [D, 128], BF16, kind="ExternalInput").ap()
    bq = nc.dram_tensor("bq", [128, 1], F32, kind="ExternalInput").ap()
    bk = nc.dram_tensor("bk", [128, 1], F32, kind="ExternalInput").ap()
    bv = nc.dram_tensor("bv", [1, 128], BF16, kind="ExternalInput").ap()
    wp = nc.dram_tensor("wp", [D, D], BF16, kind="ExternalInput").ap()
    bp = nc.dram_tensor("bp", [1, D], BF16, kind="ExternalInput").ap()
    maskd = nc.dram_tensor("maskd", [128, 2 * QG], BF16, kind="ExternalInput").ap()
    ones1 = nc.dram_tensor("ones1", [1, 128], BF16, kind="ExternalInput").ap()
    out = nc.dram_tensor("out", [QL, D], F32, kind="ExternalOutput").ap()

    with tile.TileContext(nc) as tc:
        _body(nc, tc, [xt0, xt1], wq, wk, wv, bq, bk, bv, wp, bp, maskd,
              ones1, out)
    nc.compile()
    return nc


class _Ctx:
    pass


def _body(nc, tc, xt, wq, wk, wv, bq, bk, bv, wp, bp, maskd, ones1, out):
    with (
        tc.tile_pool(name="const", bufs=1) as const_p,
        tc.tile_pool(name="w", bufs=1) as w_p,
        tc.tile_pool(name="qkv", bufs=1) as qkv_p,
        tc.tile_pool(name="xtc", bufs=2) as xtc_p,
        tc.tile_pool(name="pt", bufs=6) as p_p,
        tc.tile_pool(name="ys", bufs=16) as ys_p,
        tc.tile_pool(name="rec", bufs=4) as rec_p,
        tc.tile_pool(name="outp", bufs=3) as out_p,
        tc.tile_pool(name="psum", bufs=5, space="PSUM") as psum,
        tc.tile_pool(name="opsum", bufs=2, space="PSUM") as opsum,
        tc.tile_pool(name="rpsum", bufs=1, space="PSUM") as rpsum,
        tc.tile_pool(name="dram", bufs=1, space="DRAM") as dram,
    ):
        # ---------------- constants (tiles now, DMAs emitted just after
        # the first x-chunk DMA so phase 1 starts immediately) ----------
        ones_s = const_p.tile([1, 128], BF16)
        mask_s = const_p.tile([128, 2 * QG], BF16)
        bq_s = const_p.tile([128, 1], F32)
        bk_s = const_p.tile([128, 1], F32)
        bv_s = const_p.tile([1, 128], BF16)
        bp_s = const_p.tile([1, D], BF16)

        def emit_const_dmas():
            nc.sync.dma_start(ones_s[:], ones1[:])
            nc.sync.dma_start(bq_s[:], bq[:])
            nc.sync.dma_start(bk_s[:], bk[:])
            nc.sync.dma_start(bv_s[:], bv[:])
            nc.sync.dma_start(mask_s[:], maskd[:])
            nc.sync.dma_start(bp_s[:], bp[:])

        # ---------------- weights (one DMA each, gather-rearranged) -------
        wq_s = w_p.tile([128, D], BF16)   # [d-chunk part, kc*128 + qcol]
        wk_s = w_p.tile([128, D], BF16)
        wv_s = w_p.tile([128, D], BF16)
        for ws, wsrc in ((wq_s, wq), (wk_s, wk), (wv_s, wv)):
            nc.sync.dma_start(
                ws[:].rearrange("p (c e) -> p c e", c=NKC),
                wsrc[:].rearrange("(c p) e -> p c e", p=128),
            )

        cx = _Ctx()
        cx.qt = [[qkv_p.tile([128, TCH], BF16, tag=f"qt{b}_{t}",
                             name=f"qt{b}_{t}") for t in range(NTC)]
                 for b in range(B)]
        cx.kt = [[qkv_p.tile([128, TCH], BF16, tag=f"kt{b}_{t}",
                             name=f"kt{b}_{t}") for t in range(NTC)]
                 for b in range(B)]
        cx.v2 = [[qkv_p.tile([128, 4 * 130], BF16, tag=f"v{b}_{t}",
                             name=f"v{b}_{t}") for t in range(NTC)]
                 for b in range(B)]
        for b in range(B):
            for t in range(NTC):
                v4 = cx.v2[b][t][:].rearrange("p (t h e) -> p t h e",
                                              h=2, e=65)
                nc.vector.memset(v4[:, :, :, 64:65], 1.0)
        cx.ytn = [qkv_p.tile([128, S], BF16, tag=f"ytn{b}", name=f"ytn{b}")
                  for b in range(B)]
        cx.xc = {}

        # ---------- QKV sub-unit emitters ----------
        def qkv_dma(b, tcn):
            xc = xtc_p.tile([128, NKC * TCH], BF16, tag="x",
                            name=f"x{b}_{tcn}")
            nc.sync.dma_start(
                xc[:].rearrange("p (c t) -> p c t", c=NKC),
                xt[b][:].rearrange("(c p) t -> p c t", p=128)[
                    :, :, tcn * TCH:(tcn + 1) * TCH],
            )
            cx.xc[(b, tcn)] = xc

        def qkv_q(b, tcn):
            xc = cx.xc[(b, tcn)]
            ps = psum.tile([128, TCH], F32, tag="ps", name=f"q{b}_{tcn}")
            for kc in range(NKC):
                nc.tensor.matmul(
                    ps[:], wq_s[:, kc * 128:(kc + 1) * 128],
                    xc[:, kc * TCH:(kc + 1) * TCH],
                    start=(kc == 0), stop=(kc == NKC - 1),
                )
            if b == 0:   # batch 0: ACT is idle during this phase
                nc.scalar.activation(cx.qt[b][tcn][:], ps[:], AF.Identity,
                                     bias=bq_s[:])
            else:
                nc.vector.tensor_scalar(
                    cx.qt[b][tcn][:], ps[:], bq_s[:], None, ALU.add)

        def qkv_k(b, tcn):
            xc = cx.xc[(b, tcn)]
            ps = psum.tile([128, TCH], F32, tag="ps", name=f"k{b}_{tcn}")
            for kc in range(NKC):
                nc.tensor.matmul(
                    ps[:], wk_s[:, kc * 128:(kc + 1) * 128],
                    xc[:, kc * TCH:(kc + 1) * TCH],
                    start=(kc == 0), stop=(kc == NKC - 1),
                )
            if b == 0:
                nc.scalar.activation(cx.kt[b][tcn][:], ps[:], AF.Identity,
                                     bias=bk_s[:])
            else:
                nc.vector.tensor_scalar(
                    cx.kt[b][tcn][:], ps[:], bk_s[:], None, ALU.add)

        def qkv_v(b, tcn, vt):
            xc = cx.xc[(b, tcn)]
            ps = psum.tile([128, 512], F32, tag="ps", name=f"v{b}_{tcn}_{vt}")
            for kc in range(NKC):
                nc.tensor.matmul(
                    ps[:, 0:128],
                    xc[:, kc * TCH + vt * 128:kc * TCH + (vt + 1) * 128],
                    wv_s[:, kc * 128:(kc + 1) * 128],
                    start=(kc == 0), stop=False,
                )
            # fold the per-column v-bias in as a K=1 rank-1 update
            nc.tensor.matmul(
                ps[:, 0:128], ones_s[:], bv_s[:], start=False, stop=True,
            )
            v4 = cx.v2[b][tcn][:].rearrange("p (t h e) -> p t h e",
                                            h=2, e=65)
            if b == 0:
                nc.scalar.activation(
                    v4[:, vt, :, 0:64],
                    ps[:, 0:128].rearrange("p (h e) -> p h e", e=64),
                    AF.Copy,
                )
            else:
                nc.vector.tensor_copy(
                    v4[:, vt, :, 0:64],
                    ps[:, 0:128].rearrange("p (h e) -> p h e", e=64),
                )

        def qkv_units(b):
            units = []
            for tcn in range(NTC):
                units.append(lambda b=b, t=tcn: qkv_dma(b, t))
                units.append(lambda b=b, t=tcn: qkv_q(b, t))
                units.append(lambda b=b, t=tcn: qkv_k(b, t))
                for vt in range(4):
                    units.append(lambda b=b, t=tcn, v=vt: qkv_v(b, t, v))
            return deque(units)

        # ---------- attention stream (skewed: PV one tile behind) ----------
        class AttStream:
            def __init__(self, b, hh):
                self.b, self.hh = b, hh
                self.e0 = hh * 64
                self.ys = [ys_p.tile([65, 2 * QG], F32, tag="ys",
                                     name=f"ys{b}_{hh}_{gp}")
                           for gp in range(NQG // 2)]
                self.rec = rec_p.tile([1, S], BF16, tag="rec",
                                      name=f"rec{b}_{hh}")
                self.tiles = deque()
                for g in range(NQG):
                    npr = g + 1
                    # diagonal tile first: its extra mask hop hides behind
                    # the remaining full tiles' PV work
                    order = [npr - 1] + list(range(npr - 1))
                    for i, pr in enumerate(order):
                        self.tiles.append(
                            (g, pr, pr == npr - 1, i == 0, i == npr - 1))
                self.prev = None
                self.ops = {}
                self.gps_done = 0

            @property
            def has(self):
                return bool(self.tiles) or self.prev is not None

            def _emit_qk(self, g, pr, diag):
                b, hh, e0 = self.b, self.hh, self.e0
                q0 = g * QG
                sp = psum.tile([128, 2 * QG], F32, tag="ps",
                               name=f"s{b}_{hh}_{g}_{pr}")
                pt = p_p.tile([128, 2 * QG], BF16, tag="pt",
                              name=f"p{b}_{hh}_{g}_{pr}")
                for kk in range(2):
                    kvc = 2 * pr + kk
                    nc.tensor.matmul(
                        sp[:, kk * QG:(kk + 1) * QG],
                        cx.kt[b][kvc // 4][e0:e0 + 64,
                                           (kvc % 4) * 128:
                                           (kvc % 4 + 1) * 128],
                        cx.qt[b][g // 2][e0:e0 + 64,
                                         (q0 % TCH):(q0 % TCH) + QG],
                        start=True, stop=True,
                    )
                nc.scalar.activation(pt[:], sp[:], AF.Exp)
                if diag:
                    nc.vector.tensor_tensor(pt[:], pt[:], mask_s[:],
                                            ALU.mult)
                return pt

            def _emit_pv(self):
                g, pr, diag, first, last, ptp = self.prev
                b, hh = self.b, self.hh
                gp, gh = g // 2, g % 2
                if gp not in self.ops:
                    self.ops[gp] = opsum.tile([65, 2 * QG], F32, tag="o",
                                              name=f"o{b}_{hh}_{gp}")
                op = self.ops[gp]
                for kk in range(2):
                    kvc = 2 * pr + kk
                    v4 = cx.v2[b][kvc // 4][:].rearrange(
                        "p (t h e) -> p t h e", h=2, e=65)
                    nc.tensor.matmul(
                        op[:, gh * QG:(gh + 1) * QG],
                        v4[:, kvc % 4, hh, :],
                        ptp[:, kk * QG:(kk + 1) * QG],
                        start=(first and kk == 0),
                        stop=(last and kk == 1),
                    )
                if last and gh == 1:
                    nc.vector.tensor_copy(self.ys[gp][:], op[:])
                    del self.ops[gp]
                    self.gps_done += 1
                self.prev = None

            def step(self):
                """Emit QK+exp for the next tile, then PV for the previous."""
                if self.tiles:
                    g, pr, diag, first, last = self.tiles.popleft()
                    pt = self._emit_qk(g, pr, diag)
                    if self.prev is not None:
                        self._emit_pv()
                    self.prev = (g, pr, diag, first, last, pt)
                elif self.prev is not None:
                    self._emit_pv()

            def norm_q(self, m):
                """Normalize token slice [512m, 512m+512) (= ys tile m)."""
                b, hh, e0 = self.b, self.hh, self.e0
                rec = self.rec
                with nc.allow_low_precision(reason="bf16 softmax denom"):
                    nc.vector.reciprocal(
                        rec[:, m * 512:(m + 1) * 512],
                        self.ys[m][64:65, :])
                rp = rpsum.tile([64, 512], F32, tag="rp",
                                name=f"rp{b}_{hh}_{m}")
                nc.tensor.matmul(
                    rp[:], ones_s[:, 0:64],
                    rec[:, m * 512:(m + 1) * 512],
                    start=True, stop=True,
                )
                nc.vector.tensor_tensor(
                    cx.ytn[b][e0:e0 + 64, m * 512:(m + 1) * 512],
                    self.ys[m][0:64, :],
                    rp[:], ALU.mult,
                )

        a2a_in = dram.tile([8 * 128, QL], BF16)
        a2a_out = dram.tile([8 * 128, QL], BF16)

        def a2a_send(b, m):
            j = 4 * b + m
            nc.sync.dma_start(
                a2a_in[j * 128:(j + 1) * 128, :],
                cx.ytn[b][:, m * QL:(m + 1) * QL],
            )

        # ================= emission schedule =================
        # phase 1: QKV(b0) with early b0 attention tiles as they unblock
        s00, s01 = AttStream(0, 0), AttStream(0, 1)

        def tcn_req(st):
            g, pr = st.tiles[0][0], st.tiles[0][1]
            return max(g // 2, (2 * pr + 1) // 4)

        b0_units = qkv_units(0)
        b0_units.popleft()()      # x(b0, tcn0) DMA first
        emit_const_dmas()
        done_tcn0 = -1
        unit_i = 1
        while b0_units:
            b0_units.popleft()()
            unit_i += 1
            if unit_i % 7 == 0:
                done_tcn0 += 1   # a full tcn group (dma+q+k+4v) emitted
            for st in (s00, s01):
                if st.tiles and tcn_req(st) <= done_tcn0:
                    st.step()
        # phase 2: rest of attention(b0) + QKV(b1) interleave
        b1_units = qkv_units(1)
        tilesteps = 0
        while s00.has or s01.has:
            for st in (s00, s01):
                if st.has:
                    st.step()
                    tilesteps += 1
                    if tilesteps % 3 == 0 and b1_units:
                        b1_units.popleft()()
        while b1_units:
            b1_units.popleft()()
        # wp load: no deps; DMA queue is idle from here until the a2a sends
        wp_s = w_p.tile([128, NKC * D], BF16, tag="wp", name="wp")
        nc.sync.dma_start(
            wp_s[:].rearrange("p (c e) -> p c e", c=NKC),
            wp[:].rearrange("(c p) e -> p c e", p=128),
        )
        # phase 3: attention(b1); batch-0 norms+sends and progressive
        # batch-1 norms+sends interleave as their ys tiles complete
        s10, s11 = AttStream(1, 0), AttStream(1, 1)
        b0_norm = deque(
            [(s00, 0), (s01, 0), (0, 0),
             (s00, 1), (s01, 1), (0, 1),
             (s00, 2), (s01, 2), (0, 2),
             (s00, 3), (s01, 3), (0, 3)]
        )
        sent1 = 0
        steps = 0
        while s10.has or s11.has:
            for st in (s10, s11):
                if st.has:
                    st.step()
                    steps += 1
                    if steps % 4 == 0 and b0_norm:
                        job = b0_norm.popleft()
                        if isinstance(job[0], AttStream):
                            job[0].norm_q(job[1])
                        else:
                            a2a_send(0, job[1])
            while (sent1 < 3
                   and min(s10.gps_done, s11.gps_done) > sent1):
                s10.norm_q(sent1)
                s11.norm_q(sent1)
                a2a_send(1, sent1)
                sent1 += 1
        while b0_norm:
            job = b0_norm.popleft()
            if isinstance(job[0], AttStream):
                job[0].norm_q(job[1])
            else:
                a2a_send(0, job[1])
        # phase 4: last batch-1 slice, exchange, output projection
        for m in range(sent1, 4):
            s10.norm_q(m)
            s11.norm_q(m)
            a2a_send(1, m)

        nc.gpsimd.collective_compute(
            "AllToAll",
            ALU.bypass,
            replica_groups=[[0, 1, 2, 3, 4, 5, 6, 7]],
            ins=[a2a_in[:]],
            outs=[a2a_out[:]],
        )
        yg = qkv_p.tile([128, 8 * QL], BF16, tag="yg", name="yg")
        for r in range(8):
            nc.sync.dma_start(yg[:, r * QL:(r + 1) * QL],
                              a2a_out[r * 128:(r + 1) * 128, :])

        for tc4 in range(4):
            for n2 in range(2):
                ps = psum.tile([128, 512], F32, tag="ps",
                               name=f"op{tc4}_{n2}")
                for r in range(8):
                    nc.tensor.matmul(
                        ps[:],
                        yg[:, r * QL + tc4 * 128:r * QL + (tc4 + 1) * 128],
                        wp_s[:, r * D + n2 * 512:r * D + (n2 + 1) * 512],
                        start=(r == 0), stop=False,
                    )
                nc.tensor.matmul(
                    ps[:], ones_s[:], bp_s[:, n2 * 512:(n2 + 1) * 512],
                    start=False, stop=True,
                )
                ot = out_p.tile([128, 512], F32, tag="ot",
                                name=f"ot{tc4}_{n2}")
                nc.scalar.activation(ot[:], ps[:], AF.Copy)
                nc.sync.dma_start(
                    out[tc4 * 128:(tc4 + 1) * 128,
                        n2 * 512:(n2 + 1) * 512],
                    ot[:],
                )


def _host_inputs(x, w_attn, b_attn, w_proj, b_proj):
    """Build the 8 per-core input maps."""
    x = np.asarray(x, np.float32)
    w_attn = np.asarray(w_attn, np.float32)
    b_attn = np.asarray(b_attn, np.float32)
    w_proj = np.asarray(w_proj, np.float32)
    b_proj = np.asarray(b_proj, np.float32)

    xt0 = np.ascontiguousarray(x[0].T).astype(NPBF16)
    xt1 = np.ascontiguousarray(x[1].T).astype(NPBF16)
    wpp = np.ascontiguousarray(w_proj).astype(NPBF16)
    bp1 = b_proj.reshape(1, D).astype(NPBF16)
    ones1 = np.ones((1, 128), NPBF16)

    # diagonal-pair 0/1 mask: cols [0:256] kv-offset 0..127, [256:512] 128..255
    p = np.arange(128)[:, None]
    q = np.arange(QG)[None, :]
    maskd = np.concatenate(
        [(p <= q).astype(np.float32), (p + 128 <= q).astype(np.float32)],
        axis=1,
    ).astype(NPBF16)

    in_maps = []
    for c in range(8):
        c0 = 128 * c
        wqc = (w_attn[:, c0:c0 + 128] * SCALE).astype(NPBF16)
        wkc = w_attn[:, D + c0:D + c0 + 128].astype(NPBF16)
        wvc = w_attn[:, 2 * D + c0:2 * D + c0 + 128].astype(NPBF16)
        bqc = (b_attn[c0:c0 + 128] * SCALE).astype(np.float32).reshape(128, 1)
        bkc = b_attn[D + c0:D + c0 + 128].astype(np.float32).reshape(128, 1)
        bvc = b_attn[2 * D + c0:2 * D + c0 + 128].reshape(1, 128).astype(NPBF16)
        in_maps.append(
            {
                "xt0": xt0, "xt1": xt1,
                "wq": wqc, "wk": wkc, "wv": wvc,
                "bq": bqc, "bk": bkc, "bv": bvc,
                "wp": wpp, "bp": bp1,
                "maskd": maskd, "ones1": ones1,
            }
        )
    return in_maps


def _assemble_full(outs):
    full = np.empty((B, S, D), np.float32)
    for c in range(8):
        b, cq = c // 4, c % 4
        full[b, cq * QL:(cq + 1) * QL] = outs[c]
    return full


def kernel(x, w_attn, b_attn, w_proj, b_proj):
    if "nc" not in _CACHED:
        _CACHED["nc"] = build_nc()
    nc = _CACHED["nc"]
    in_maps = _host_inputs(x, w_attn, b_attn, w_proj, b_proj)
    res = run_bass_kernel_spmd(nc, in_maps, core_ids=list(range(8)))
    _CACHED["last_results"] = res
    outs = [res.results[c]["out"] for c in range(8)]
    return _assemble_full(outs)
